# revision 38
# baseline (speedup 1.0000x reference)
"""DeepseekV3 MLA attention forward on 8 Trainium2 NeuronCores.

Sharding: core c -> batch c//4, head group c%4 (4 of 16 heads) for the
attention stages; the low-rank down-projections (stage A) are token-sharded
across the 4 cores of each batch group (core g owns token tiles
{g, 4+g, 8+g, 12+g}) and the normalized latents are exchanged with 4
pipelined AllGather collectives, one per contiguous 512-token block.

Stage A computes transposed ([feature, token]-major) outputs directly so no
DMA transposes are needed: out = W_chunk^T @ h_chunk with tokens on the
moving free dim.  RMSNorm in this layout does the cross-partition
sum-of-squares with a ones-vector matmul, and broadcasts the per-token
1/rms across partitions with a rank-1 matmul.

Stages B/C/D are head-sharded as before, emitted per 512-token chunk
(B(j) -> C(qc=j) -> D(j)) so the tensor engine consumes gathered chunks as
they land.  Host sums the 4 partial wo-projections per batch.
"""

import math

import numpy as np
import ml_dtypes

import concourse.bass as bass
import concourse.tile as tile
from concourse.tile import add_dep_helper
import concourse.mybir as mybir
from concourse import bacc
from concourse.bass_utils import run_bass_kernel_spmd

BF16 = mybir.dt.bfloat16
F32 = mybir.dt.float32
AF = mybir.ActivationFunctionType

# ---- model config (hardcoded to match the problem spec) ----
HIDDEN = 2048
N_HEADS = 16
Q_LORA = 1536
KV_LORA = 512
NOPE = 128
ROPE = 64
VHD = 128
QHD = NOPE + ROPE  # 192
BASE = 10000.0
SCALE = 40.0
ORIG_MAX = 4096
BETA_FAST = 32
BETA_SLOW = 1
EPS = 1e-6
B = 2
S = 2048

N_CORES = 8
HL = 4          # heads per core
P = 128
TT = S // P     # 16 token tiles
QC = S // 512   # 4 query chunks of 512
KT = S // P     # 16 key tiles
NLOC = 4        # local token tiles per core (sequence shard)

KH = HIDDEN // P    # 16
KQ = Q_LORA // P    # 12
KKV = KV_LORA // P  # 4
NSLOT = KQ + KKV + 1  # 17 feature slots of 128 in the gather payload

_m = 0.1 * math.log(SCALE) + 1.0
SOFT_SCALE = (QHD ** -0.5) * _m * _m

REPLICA_GROUPS = [[0, 1, 2, 3], [4, 5, 6, 7]]


def _yarn_cos_sin(seq_len):
    dim = ROPE
    ar = np.arange(0, dim, 2, dtype=np.float32)
    freq_extra = 1.0 / BASE ** (ar / dim)
    freq_inter = 1.0 / (SCALE * BASE ** (ar / dim))
    low = math.floor(dim * math.log(ORIG_MAX / (BETA_FAST * 2 * math.pi)) / (2 * math.log(BASE)))
    high = math.ceil(dim * math.log(ORIG_MAX / (BETA_SLOW * 2 * math.pi)) / (2 * math.log(BASE)))
    low, high = max(low, 0), min(high, dim - 1)
    denom = (high - low) if high != low else 0.001
    ramp = np.clip((np.arange(dim // 2, dtype=np.float32) - low) / denom, 0.0, 1.0)
    inv_freq_mask = 1.0 - ramp
    inv_freq = freq_inter * (1.0 - inv_freq_mask) + freq_extra * inv_freq_mask
    t = np.arange(seq_len, dtype=np.float32)
    freqs = np.outer(t, inv_freq)
    emb = np.concatenate([freqs, freqs], axis=-1)
    # mscale ratio is 1.0 for this config
    return np.cos(emb).astype(np.float32), np.sin(emb).astype(np.float32)


_PERM64 = np.concatenate([np.arange(0, 64, 2), np.arange(1, 64, 2)])


def _bf16(x):
    return np.ascontiguousarray(x.astype(ml_dtypes.bfloat16))


def _emit_stage_a(nc, tc, contribs, gatheds, hT_loc, wqaT, wkvaT,
                  cos_fm, sin_fm_s, eps_sb, ones_b, ones_f):
    """Token-sharded down-projections with transposed outputs + AllGather."""
    with (
        tc.tile_pool(name="wA", bufs=1) as wA,
        tc.tile_pool(name="stgp", bufs=2) as stgp,
        tc.tile_pool(name="sqp", bufs=2) as sqp,
        tc.tile_pool(name="scrA", bufs=4) as scrA,
        tc.tile_pool(name="psA", bufs=4, space="PSUM") as psA,
        tc.tile_pool(name="psK", bufs=1, space="PSUM") as psK,
        tc.tile_pool(name="psN", bufs=1, space="PSUM") as psN,
    ):
        wqa_sb = wA.tile([P, KH, Q_LORA], BF16, tag="wqa")
        wkva_sb = wA.tile([P, KH, KV_LORA + ROPE], BF16, tag="wkva")
        hl_sb = wA.tile([P, KH, NLOC * P], BF16, tag="hl")
        cosf = wA.tile([64, NLOC * P], BF16, tag="cosf")
        sinf = wA.tile([64, NLOC * P], BF16, tag="sinf")
        # weights stream in 512-column blocks matching the f-group
        # consumption order, so the PE starts as soon as block 0 lands
        nc.sync.dma_start(hl_sb[:], hT_loc[:].rearrange("(k p) t -> p k t", p=P))
        for fb in range(Q_LORA // 512):
            nc.sync.dma_start(
                wqa_sb[:, :, fb * 512:(fb + 1) * 512],
                wqaT[:, fb * 512:(fb + 1) * 512].rearrange(
                    "(k p) c -> p k c", p=P))
        nc.sync.dma_start(
            wkva_sb[:, :, 0:KV_LORA],
            wkvaT[:, 0:KV_LORA].rearrange("(k p) c -> p k c", p=P))
        nc.sync.dma_start(
            wkva_sb[:, :, KV_LORA:],
            wkvaT[:, KV_LORA:].rearrange("(k p) c -> p k c", p=P))
        nc.sync.dma_start(cosf[:], cos_fm[:])
        nc.sync.dma_start(sinf[:], sin_fm_s[:])

        NG = (KQ + KKV) // 4  # 4 f-groups per token tile

        def emit_group_mms(j, fg, ps):
            tok = slice(j * P, (j + 1) * P)
            for fi in range(4):
                f = fg * 4 + fi
                if f < KQ:
                    w, fo = wqa_sb, f * P
                else:
                    w, fo = wkva_sb, (f - KQ) * P
                for k in range(KH):
                    nc.tensor.matmul(ps[:, fi], w[:, k, fo:fo + P],
                                     hl_sb[:, k, tok],
                                     start=(k == 0), stop=(k == KH - 1))

        def emit_group_post(j, fg, ps, raw, sq, ssq):
            # PSUM -> SBUF copies + squares on DVE (Act stays free for Sqrt),
            # then the cross-partition sum-of-squares accumulation
            for fi in range(4):
                f = fg * 4 + fi
                nc.vector.tensor_copy(raw[:, f], ps[:, fi])
                nc.vector.tensor_mul(sq[:, f], raw[:, f], raw[:, f])
            for fi in range(4):
                f = fg * 4 + fi
                if f < KQ:
                    nc.tensor.matmul(ssq[:, 0:P], ones_b[:], sq[:, f],
                                     start=(f == 0), stop=(f == KQ - 1))
                else:
                    nc.tensor.matmul(ssq[:, P:2 * P], ones_b[:], sq[:, f],
                                     start=(f == KQ), stop=(f == KQ + KKV - 1))

        def emit_finalize(j, raw, stg, rinv):
            # broadcast 1/rms down the partitions, normalize, ship, gather
            sbc = psN.tile([P, 2, P], F32, tag="sbc", name=f"sbc{j}")
            nc.tensor.matmul(sbc[:, 0], ones_f[:], rinv[:, 0:P],
                             start=True, stop=True)
            nc.tensor.matmul(sbc[:, 1], ones_f[:], rinv[:, P:2 * P],
                             start=True, stop=True)
            for f in range(KQ):
                nc.vector.tensor_mul(stg[:, f], raw[:, f], sbc[:, 0])
            for f in range(KKV):
                nc.vector.tensor_mul(stg[:, KQ + f], raw[:, KQ + f], sbc[:, 1])
            nc.sync.dma_start(
                contribs[j][:].rearrange("p (s t) -> p s t", t=P), stg[:])
            nc.gpsimd.collective_compute(
                "AllGather",
                mybir.AluOpType.bypass,
                replica_groups=REPLICA_GROUPS,
                ins=[contribs[j][:].opt()],
                outs=[gatheds[j][:].opt()],
            )

        pending = None
        for j in range(NLOC):
            tok = slice(j * P, (j + 1) * P)
            stg = stgp.tile([P, NSLOT, P], BF16, tag="stg", name=f"stg{j}")
            sq = sqp.tile([P, KQ + KKV, P], BF16, tag="sq", name=f"sq{j}")
            raw = sqp.tile([P, KQ + KKV, P], BF16, tag="raw", name=f"raw{j}")
            ssq = psN.tile([1, 2 * P], F32, tag="ssq", name=f"ssq{j}")
            nc.vector.memset(stg[64:128, KQ + KKV], 0.0)
            # group matmuls with post-processing lagged one group so the PE
            # never waits on the DVE squares; the previous tile's finalize
            # (which waits on an Act/DVE round-trip) is sandwiched after
            # this tile's first group
            groups = []
            for fg in range(NG):
                ps = psA.tile([P, 4, P], F32, tag="aps")
                emit_group_mms(j, fg, ps)
                groups.append(ps)
                if fg == 0 and pending is not None:
                    emit_finalize(*pending)
                    pending = None
                if fg >= 1:
                    emit_group_post(j, fg - 1, groups[fg - 1], raw, sq, ssq)
            emit_group_post(j, NG - 1, groups[NG - 1], raw, sq, ssq)

            rms = scrA.tile([1, 2 * P], F32, tag="rms")
            nc.scalar.activation(rms[:, 0:P], ssq[:, 0:P], AF.Sqrt,
                                 scale=1.0 / Q_LORA, bias=eps_sb[0:1])
            nc.scalar.activation(rms[:, P:2 * P], ssq[:, P:2 * P], AF.Sqrt,
                                 scale=1.0 / KV_LORA, bias=eps_sb[0:1])
            rinv = scrA.tile([1, 2 * P], F32, tag="rinv", name=f"rinv{j}")
            nc.vector.reciprocal(rinv[:], rms[:])

            # rope branch of ckv (no norm) keeps the PE busy while the
            # Sqrt/reciprocal round-trip completes
            psk = psK.tile([64, P], F32, tag="kps")
            for k in range(KH):
                nc.tensor.matmul(psk[:], wkva_sb[:, k, KV_LORA:], hl_sb[:, k, tok],
                                 start=(k == 0), stop=(k == KH - 1))
            tmp = scrA.tile([64, P], BF16, tag="ktmp")
            nc.vector.tensor_mul(tmp[0:32], psk[32:64], sinf[0:32, tok])
            nc.vector.tensor_mul(tmp[32:64], psk[0:32], sinf[32:64, tok])
            nc.vector.tensor_mul(stg[0:64, KQ + KKV], psk[:], cosf[:, tok])
            nc.vector.tensor_add(stg[0:64, KQ + KKV], stg[0:64, KQ + KKV], tmp[:])

            pending = (j, raw, stg, rinv)
        emit_finalize(*pending)


def _emit_b_chunk(nc, tc, pools, tensors, tcks):
    """Stage B (up-projections + q rope) for one 512-token chunk.

    All outputs are produced in their consumer layouts directly (no DMA
    transposes -- those serialize against the collectives):
      kn / q_nope / qpe_rot: [feature, token]; v: [token, feature].
    """
    workB, psB = pools
    (wqb_sb, wkvb_sb, qncn_ch, cosfS, sinfS,
     kn_sb, vaug, q_nope, qpe_rot) = tensors
    blk = slice(tcks * 512, (tcks + 1) * 512)
    first_mm = None

    # ---- B-KV: k_nope [feat, tok] ----
    for ft in range(HL):
        ps = psB.tile([P, 512], F32, tag="up")
        for k in range(KKV):
            m = nc.tensor.matmul(
                ps[:], wkvb_sb[:, k, ft * P:(ft + 1) * P],
                qncn_ch[:, :, KQ + k, :],
                start=(k == 0), stop=(k == KKV - 1))
            if first_mm is None:
                first_mm = m
        nc.scalar.copy(kn_sb[:, ft, blk], ps[:])
    # ---- B-KV: v [tok, 4 heads x 128] per token tile ----
    for tsub in range(4):
        kt = tcks * 4 + tsub
        ps = psB.tile([P, 512], F32, tag="up")
        for k in range(KKV):
            nc.tensor.matmul(
                ps[:], qncn_ch[:, tsub, KQ + k, :],
                wkvb_sb[:, k, HL * P:HL * P + HL * VHD],
                start=(k == 0), stop=(k == KKV - 1))
        for h in range(HL):
            nc.scalar.copy(vaug[:, h, kt, 0:VHD], ps[:, h * VHD:(h + 1) * VHD])

    # ---- B-Q nope [feat, tok] ----
    for ft in range(HL):
        ps = psB.tile([P, 512], F32, tag="up")
        for k in range(KQ):
            nc.tensor.matmul(
                ps[:], wqb_sb[:, k, ft * P:(ft + 1) * P],
                qncn_ch[:, :, k, :],
                start=(k == 0), stop=(k == KQ - 1))
        nc.scalar.copy(q_nope[:, ft, blk], ps[:])
    # ---- B-Q pe [feat, tok] + feature-major rope ----
    for ft in range(2):
        ps = psB.tile([P, 512], F32, tag="up")
        for k in range(KQ):
            nc.tensor.matmul(
                ps[:], wqb_sb[:, k, HL * P + ft * P:HL * P + (ft + 1) * P],
                qncn_ch[:, :, k, :],
                start=(k == 0), stop=(k == KQ - 1))
        for half in range(2):
            h = 2 * ft + half
            h0 = half * 64
            tmp = workB.tile([64, 512], BF16, tag="qtmp")
            nc.vector.tensor_mul(tmp[0:32], ps[h0 + 32:h0 + 64],
                                 sinfS[0:32, blk])
            nc.vector.tensor_mul(tmp[32:64], ps[h0:h0 + 32],
                                 sinfS[32:64, blk])
            nc.vector.tensor_mul(qpe_rot[:, h, blk], ps[h0:h0 + 64],
                                 cosfS[:, blk])
            nc.vector.tensor_add(qpe_rot[:, h, blk], qpe_rot[:, h, blk],
                                 tmp[:])
    return first_mm



def _emit_c_chunk_old(nc, tc, psS, ptp, workC, psO, psB, ones_b, ones_f, kn_sb,
                      q_nope, qpe_rot, krotT, vaug, attn_ch, mask_sb, qc):
    for h in range(HL):
        nkt = 4 * qc + 4
        pts = []
        offs = []
        for kt in range(nkt):
            diag = (kt // 4 == qc)
            off = (kt % 4) * P if diag else 0
            w = 512 - off
            ss = psS.tile([P, 512], F32, tag="ss")
            nc.tensor.matmul(
                ss[:, 0:w], kn_sb[:, h, kt * P:(kt + 1) * P],
                q_nope[:, h, qc * 512 + off:(qc + 1) * 512],
                start=True, stop=False)
            nc.tensor.matmul(
                ss[:, 0:w], krotT[:, kt * P:(kt + 1) * P],
                qpe_rot[:, h, qc * 512 + off:(qc + 1) * 512],
                start=False, stop=True)
            pt_t = ptp.tile([P, 512], BF16, tag="pt")
            nc.scalar.activation(pt_t[:, 0:w], ss[:, 0:w], AF.Exp,
                                 scale=SOFT_SCALE)
            if diag:
                nc.vector.tensor_mul(pt_t[:, 0:P], pt_t[:, 0:P],
                                     mask_sb[:, 384:384 + P])
            pts.append(pt_t)
            offs.append(off)
        for qsub in range(4):
            qt = 4 * qc + qsub
            po = psO.tile([P, 512], F32, tag="po")
            for kt in range(qt + 1):
                o = qsub * P - offs[kt]
                last_mm = nc.tensor.matmul(
                    po[:, 0:VHD + 1], pts[kt][:, o:o + P],
                    vaug[:, h, kt, 0:VHD + 1],
                    start=(kt == 0), stop=(kt == qt))
            rd = workC.tile([P, 1], F32, tag="rd")
            nc.vector.reciprocal(rd[:], po[:, VHD:VHD + 1])
            at = workC.tile([P, VHD], BF16, tag="at")
            nc.vector.tensor_scalar_mul(at[:], po[:, 0:VHD], rd[:])
            nc.sync.dma_start_transpose(
                attn_ch[:, h, qsub * P:(qsub + 1) * P], at[:])
    return last_mm


def _emit_c_chunk(nc, tc, psS, ptp, workC, psO, psB, ones_b, ones_f, kn_sb,
                  q_nope, qpe_rot, krotT, vaug, attn_ch, mask_sb, qc):
    """Stage C (attention) for query chunk qc (512 queries).

    PV runs with V as the stationary operand so the per-head output lands
    [v, q]-major (the wo-projection layout) without a DMA transpose.  The
    softmax denominator accumulates with a ones-vector matmul and is
    broadcast back down the partitions with a rank-1 matmul.
    """
    def emit_pv(po, den, h, kt, off, pt_t):
        # accumulate this key tile into every query sub-block it reaches;
        # each 128-query region is its own uniform accumulation group
        for qsub in range(off // P, 4):
            qt = 4 * qc + qsub
            o = qsub * P - off
            reg = slice(qsub * P, (qsub + 1) * P)
            nc.tensor.matmul(
                po[:, reg], vaug[:, h, kt, 0:VHD], pt_t[:, o:o + P],
                start=(kt == 0), stop=(kt == qt))
            nc.tensor.matmul(
                den[0:1, reg], ones_b[:], pt_t[:, o:o + P],
                start=(kt == 0), stop=(kt == qt))

    for h in range(HL):
        nkt = 4 * qc + 4
        po = psO.tile([P, 512], F32, tag="po", name=f"po{qc}_{h}")
        den = psB.tile([P, 512], F32, tag="up", name=f"den{qc}_{h}")
        prev = None
        for kt in range(nkt):
            diag = (kt // 4 == qc)
            off = (kt % 4) * P if diag else 0
            w = 512 - off
            ss = psS.tile([P, 512], F32, tag="ss")
            nc.tensor.matmul(
                ss[:, 0:w], kn_sb[:, h, kt * P:(kt + 1) * P],
                q_nope[:, h, qc * 512 + off:(qc + 1) * 512],
                start=True, stop=False)
            nc.tensor.matmul(
                ss[:, 0:w], krotT[:, kt * P:(kt + 1) * P],
                qpe_rot[:, h, qc * 512 + off:(qc + 1) * 512],
                start=False, stop=True)
            pt_t = ptp.tile([P, 512], BF16, tag="pt")
            nc.scalar.activation(pt_t[:, 0:w], ss[:, 0:w], AF.Exp,
                                 scale=SOFT_SCALE)
            if diag:
                nc.vector.tensor_mul(pt_t[:, 0:P], pt_t[:, 0:P],
                                     mask_sb[:, 384:384 + P])
            # PV + denominator lag one key tile so the PE never waits on Exp
            if prev is not None:
                emit_pv(po, den, h, *prev)
            prev = (kt, off, pt_t)
        emit_pv(po, den, h, *prev)
        # normalize: 1/den broadcast down partitions, applied on the way out
        dinv = workC.tile([1, 512], F32, tag="dinv")
        nc.vector.reciprocal(dinv[:], den[0:1, :])
        dbc = psB.tile([P, 512], F32, tag="up", name=f"dbc{qc}_{h}")
        nc.tensor.matmul(dbc[:], ones_f[:], dinv[:], start=True, stop=True)
        araw = workC.tile([P, 512], BF16, tag="araw")
        nc.scalar.copy(araw[:], po[:])
        nc.vector.tensor_mul(attn_ch[:, h, :], araw[:], dbc[:])


def _build_nc():
    nc = bacc.Bacc("TRN2", target_bir_lowering=False, debug=False,
                   num_devices=N_CORES)

    hT_loc = nc.declare_dram_parameter("hT_loc", [HIDDEN, NLOC * P], BF16,
                                       isOutput=False)
    wqaT = nc.declare_dram_parameter("wqaT", [HIDDEN, Q_LORA], BF16, isOutput=False)
    wkvaT = nc.declare_dram_parameter("wkvaT", [HIDDEN, KV_LORA + ROPE], BF16,
                                      isOutput=False)
    wqbT = nc.declare_dram_parameter("wqbT", [Q_LORA, HL * QHD], BF16, isOutput=False)
    wkvbT = nc.declare_dram_parameter("wkvbT", [KV_LORA, HL * (NOPE + VHD)], BF16,
                                      isOutput=False)
    woT = nc.declare_dram_parameter("woT", [HL * VHD, HIDDEN], BF16, isOutput=False)
    cos_fS = nc.declare_dram_parameter("cos_fS", [ROPE, S], BF16, isOutput=False)
    sin_fS = nc.declare_dram_parameter("sin_fS", [ROPE, S], BF16, isOutput=False)
    cos_fm = nc.declare_dram_parameter("cos_fm", [ROPE, NLOC * P], BF16,
                                       isOutput=False)
    sin_fm_s = nc.declare_dram_parameter("sin_fm_s", [ROPE, NLOC * P], BF16,
                                         isOutput=False)
    masks = nc.declare_dram_parameter("masks", [P, 896], BF16, isOutput=False)
    outT = nc.declare_dram_parameter("outT", [HIDDEN, S], F32, isOutput=True)

    with tile.TileContext(nc) as tc:
        with (
            tc.tile_pool(name="glob", bufs=1) as pp,
            tc.tile_pool(name="dram", bufs=1, space="DRAM") as dram,
        ):
            krotT = pp.tile([64, S], BF16, tag="krotT")
            cosfS = pp.tile([64, S], BF16, tag="cosfS")
            sinfS = pp.tile([64, S], BF16, tag="sinfS")
            ones_b = pp.tile([P, 1], BF16, tag="ones_b")
            ones_f = pp.tile([1, P], F32, tag="ones_f")
            nc.vector.memset(ones_b[:], 1.0)
            nc.vector.memset(ones_f[:], 1.0)
            eps_sb = pp.tile([P, 1], F32, tag="eps")
            wkvb_sb = pp.tile([P, KKV, HL * (NOPE + VHD)], BF16, tag="wkvb")
            mask_sb = pp.tile([P, 896], BF16, tag="mask_sb")
            nc.vector.memset(eps_sb[:], EPS)
            # B/C-stage constants load on the Act DMA queue (idle during
            # stage A) so the stage-A weight stream owns the SP queue
            nc.scalar.dma_start(mask_sb[:], masks[:])
            nc.scalar.dma_start(cosfS[:], cos_fS[:])
            nc.scalar.dma_start(sinfS[:], sin_fS[:])
            nc.scalar.dma_start(
                wkvb_sb[:], wkvbT[:].rearrange("(k p) c -> p k c", p=P))

            contribs = []
            gatheds = []
            for j in range(NLOC):
                contribs.append(dram.tile([P, NSLOT * P], BF16,
                                          tag=f"contrib{j}",
                                          name=f"contrib{j}"))
                gatheds.append(dram.tile([4, P, NSLOT * P], BF16,
                                         tag=f"gathered{j}",
                                         name=f"gathered{j}"))

            # ====== Stage A (token-sharded + AllGather) ======================
            _emit_stage_a(nc, tc, contribs, gatheds, hT_loc, wqaT, wkvaT,
                          cos_fm, sin_fm_s, eps_sb, ones_b, ones_f)

            # ====== Stages B + C + D, pipelined per 512-token chunk ==========
            with (
                tc.tile_pool(name="outs", bufs=1) as outs,
                tc.tile_pool(name="chunk", bufs=2) as chunkp,
                tc.tile_pool(name="wBQ", bufs=1) as wBQ,
                tc.tile_pool(name="workB", bufs=2) as workB,
                tc.tile_pool(name="wD", bufs=1) as wD,
                tc.tile_pool(name="ptp", bufs=16) as ptp,
                tc.tile_pool(name="workC", bufs=4) as workC,
                tc.tile_pool(name="obp", bufs=2) as obp,
                tc.tile_pool(name="psB", bufs=2, space="PSUM") as psB,
                tc.tile_pool(name="psS", bufs=2, space="PSUM") as psS,
                tc.tile_pool(name="psO", bufs=2, space="PSUM") as psO,
                tc.tile_pool(name="psD", bufs=2, space="PSUM") as psD,
            ):
                kn_sb = outs.tile([P, HL, S], BF16, tag="kn_sb")
                vaug = outs.tile([P, HL, KT, VHD + 16], BF16, tag="vaug")
                q_nope = outs.tile([P, HL, S], BF16, tag="q_nope")
                qpe_rot = outs.tile([64, HL, S], BF16, tag="qpe_rot")
                nc.vector.memset(vaug[:, :, :, VHD], 1.0)

                wqb_sb = wBQ.tile([P, KQ, HL * QHD], BF16, tag="wqb")
                nc.sync.dma_start(
                    wqb_sb[:], wqbT[:].rearrange("(k p) c -> p k c", p=P))
                wo_sb = wD.tile([P, HL, HIDDEN], BF16, tag="wo")
                nc.sync.dma_start(
                    wo_sb[:], woT[:].rearrange("(k p) c -> p k c", p=P))

                def emit_d_chunk(tcks, attn_ch):
                    for og in range(HIDDEN // P // 4):
                        ob = obp.tile([P, 4, 512], F32, tag="ob")
                        for oi in range(4):
                            ot = og * 4 + oi
                            ps = psD.tile([P, 512], F32, tag="wops")
                            for k in range(HL):
                                nc.tensor.matmul(
                                    ps[:], wo_sb[:, k, ot * P:(ot + 1) * P],
                                    attn_ch[:, k, :],
                                    start=(k == 0), stop=(k == HL - 1))
                            nc.scalar.copy(ob[:, oi], ps[:])
                        nc.sync.dma_start(
                            outT[og * 512:(og + 1) * 512,
                                 tcks * 512:(tcks + 1) * 512].rearrange(
                                     "(o p) c -> p o c", p=P), ob[:])

                b_pools = (workB, psB)
                attn_chs = []
                prev_c_last = None
                for tcks in range(4):
                    # unpack gathered block tcks from DRAM into SBUF: two
                    # merged DMAs on the Act queue (idle until B(tcks) runs).
                    # Layout is source-block-major: [p, g, slot, t]
                    qncn_ch = chunkp.tile([P, 4, KQ + KKV, P], BF16,
                                          tag="qncn_ch", name=f"qncn{tcks}")
                    g = gatheds[tcks]
                    nc.gpsimd.dma_start(
                        qncn_ch[:].rearrange("p g s t -> p g (s t)"),
                        g[:, :, 0:(KQ + KKV) * P].rearrange("g p c -> p g c"))
                    nc.gpsimd.dma_start(
                        krotT[:, 4 * tcks * P:(4 * tcks + 4) * P].rearrange(
                            "p (g t) -> p g t", g=4),
                        g[:, 0:64, (KQ + KKV) * P:].rearrange("g p t -> p g t"))

                    b_tensors = (wqb_sb, wkvb_sb, qncn_ch, cosfS, sinfS,
                                 kn_sb, vaug, q_nope, qpe_rot)
                    b_first = _emit_b_chunk(nc, tc, b_pools, b_tensors, tcks)
                    if prev_c_last is not None:
                        # PE-stream-only ordering: the static scheduler does
                        # not model collective latency and otherwise hoists
                        # later chunks' B matmuls ahead of this chunk's C
                        add_dep_helper(b_first.ins, prev_c_last.ins,
                                       sync=False,
                                       reason="chunk pipeline order")

                    attn_ch = chunkp.tile([P, HL, 512], BF16, tag="attn_ch",
                                          name=f"attn{tcks}")
                    attn_chs.append(attn_ch)
                    prev_c_last = _emit_c_chunk_old(
                        nc, tc, psS, ptp, workC, psO, psB,
                        ones_b, ones_f, kn_sb, q_nope, qpe_rot,
                        krotT, vaug, attn_ch, mask_sb, tcks)
                    # wo-projection lags one chunk so its input transposes
                    # complete while the next chunk's B/C matmuls run
                    if tcks >= 1:
                        emit_d_chunk(tcks - 1, attn_chs[tcks - 1])
                emit_d_chunk(3, attn_chs[3])

    nc.compile()
    return nc


_NC_CACHE = {}
_LAST_RES = None
_LAST_IN_MAPS = None


def _get_nc(stages="ABCD"):
    if "full" not in _NC_CACHE:
        _NC_CACHE["full"] = _build_nc()
    return _NC_CACHE["full"]


def kernel(hidden_states, position_ids, wq_a, q_a_ln_w, wq_b, wkv_a, kv_a_ln_w,
           wkv_b, wo):
    hidden_states = np.asarray(hidden_states, dtype=np.float32)
    position_ids = np.asarray(position_ids)
    wq_a = np.asarray(wq_a, dtype=np.float32)
    wq_b = np.asarray(wq_b, dtype=np.float32)
    wkv_a = np.asarray(wkv_a, dtype=np.float32)
    wkv_b = np.asarray(wkv_b, dtype=np.float32)
    wo = np.asarray(wo, dtype=np.float32)
    # fold RMSNorm elementwise weights into the up-projections (exact)
    wq_b = wq_b * np.asarray(q_a_ln_w, dtype=np.float32)[None, :]
    wkv_b = wkv_b * np.asarray(kv_a_ln_w, dtype=np.float32)[None, :]
    assert hidden_states.shape == (B, S, HIDDEN)

    cos_t, sin_t = _yarn_cos_sin(S)

    # --- weight preprocessing (shared across cores in each batch group) ---
    wqbT_groups = []
    wkvbT_groups = []
    woT_groups = []
    for g in range(4):
        heads = range(4 * g, 4 * g + 4)
        rows = []
        for h in heads:
            rows.append(np.arange(h * QHD, h * QHD + NOPE))
        pe_rows = []
        for h in heads:
            pe_rows.append(h * QHD + NOPE + _PERM64)
        rows = np.concatenate(rows + pe_rows)
        wqbT_groups.append(_bf16(wq_b[rows].T))

        rows = []
        for h in heads:
            rows.append(np.arange(h * (NOPE + VHD), h * (NOPE + VHD) + NOPE))
        for h in heads:
            rows.append(np.arange(h * (NOPE + VHD) + NOPE, (h + 1) * (NOPE + VHD)))
        rows = np.concatenate(rows)
        wkvbT_groups.append(_bf16(wkv_b[rows].T))

        cols = np.concatenate([np.arange(h * VHD, (h + 1) * VHD) for h in heads])
        woT_groups.append(_bf16(wo[:, cols].T))

    wqaT = _bf16(wq_a.T)
    wkva_perm = wkv_a.copy()
    wkva_perm[KV_LORA:] = wkv_a[KV_LORA + _PERM64]
    wkvaT = _bf16(wkva_perm.T)

    x_idx = np.arange(896)[None, :]
    p_idx = np.arange(P)[:, None]
    masks = _bf16((x_idx >= 384 + p_idx).astype(np.float32))

    # --- per-batch rope tables (token-major, for stage B) ---
    batch_tabs = []
    for beta in range(B):
        pos = position_ids[beta].astype(np.int64)
        cg = cos_t[pos]          # [S, 64]
        sg = sin_t[pos]
        sin_s = np.concatenate([-sg[:, :32], sg[:, 32:]], axis=1)
        batch_tabs.append((cg, sin_s, hidden_states[beta].T))

    in_maps = []
    for c in range(N_CORES):
        beta, g = c // 4, c % 4
        cg, sin_s, hT = batch_tabs[beta]
        # local token tiles {g, 4+g, 8+g, 12+g}: 512 tokens in local order
        loc_tok = np.concatenate(
            [np.arange(P * (4 * j + g), P * (4 * j + g + 1)) for j in range(NLOC)])
        in_maps.append({
            "hT_loc": _bf16(hT[:, loc_tok]),
            "wqaT": wqaT,
            "wkvaT": wkvaT,
            "wqbT": wqbT_groups[g],
            "wkvbT": wkvbT_groups[g],
            "woT": woT_groups[g],
            "cos_fS": _bf16(cg.T),
            "sin_fS": _bf16(sin_s.T),
            "cos_fm": _bf16(cg[loc_tok].T),
            "sin_fm_s": _bf16(sin_s[loc_tok].T),
            "masks": masks,
        })

    nc = _get_nc()
    global _LAST_RES, _LAST_IN_MAPS
    _LAST_IN_MAPS = in_maps
    res = run_bass_kernel_spmd(nc, in_maps, core_ids=list(range(N_CORES)))
    _LAST_RES = res

    out = np.zeros((B, S, HIDDEN), dtype=np.float32)
    for c in range(N_CORES):
        out[c // 4] += res.results[c]["outT"].T
    return out


# revision 39
# speedup vs baseline: 1.0211x; 1.0211x over previous
"""DeepseekV3 MLA attention forward on 8 Trainium2 NeuronCores.

Sharding: core c -> batch c//4, head group c%4 (4 of 16 heads) for the
attention stages; the low-rank down-projections (stage A) are token-sharded
across the 4 cores of each batch group (core g owns token tiles
{g, 4+g, 8+g, 12+g}) and the normalized latents are exchanged with 4
pipelined AllGather collectives, one per contiguous 512-token block.

Stage A computes transposed ([feature, token]-major) outputs directly so no
DMA transposes are needed: out = W_chunk^T @ h_chunk with tokens on the
moving free dim.  RMSNorm in this layout does the cross-partition
sum-of-squares with a ones-vector matmul, and broadcasts the per-token
1/rms across partitions with a rank-1 matmul.

Stages B/C/D are head-sharded as before, emitted per 512-token chunk
(B(j) -> C(qc=j) -> D(j)) so the tensor engine consumes gathered chunks as
they land.  Host sums the 4 partial wo-projections per batch.
"""

import math

import numpy as np
import ml_dtypes

import concourse.bass as bass
import concourse.tile as tile
import concourse.mybir as mybir
from concourse import bacc
from concourse.bass_utils import run_bass_kernel_spmd

BF16 = mybir.dt.bfloat16
F32 = mybir.dt.float32
AF = mybir.ActivationFunctionType

# ---- model config (hardcoded to match the problem spec) ----
HIDDEN = 2048
N_HEADS = 16
Q_LORA = 1536
KV_LORA = 512
NOPE = 128
ROPE = 64
VHD = 128
QHD = NOPE + ROPE  # 192
BASE = 10000.0
SCALE = 40.0
ORIG_MAX = 4096
BETA_FAST = 32
BETA_SLOW = 1
EPS = 1e-6
B = 2
S = 2048

N_CORES = 8
HL = 4          # heads per core
P = 128
TT = S // P     # 16 token tiles
QC = S // 512   # 4 query chunks of 512
KT = S // P     # 16 key tiles
NLOC = 4        # local token tiles per core (sequence shard)

KH = HIDDEN // P    # 16
KQ = Q_LORA // P    # 12
KKV = KV_LORA // P  # 4
NSLOT = KQ + KKV + 1  # 17 feature slots of 128 in the gather payload

_m = 0.1 * math.log(SCALE) + 1.0
SOFT_SCALE = (QHD ** -0.5) * _m * _m

REPLICA_GROUPS = [[0, 1, 2, 3], [4, 5, 6, 7]]


def _yarn_cos_sin(seq_len):
    dim = ROPE
    ar = np.arange(0, dim, 2, dtype=np.float32)
    freq_extra = 1.0 / BASE ** (ar / dim)
    freq_inter = 1.0 / (SCALE * BASE ** (ar / dim))
    low = math.floor(dim * math.log(ORIG_MAX / (BETA_FAST * 2 * math.pi)) / (2 * math.log(BASE)))
    high = math.ceil(dim * math.log(ORIG_MAX / (BETA_SLOW * 2 * math.pi)) / (2 * math.log(BASE)))
    low, high = max(low, 0), min(high, dim - 1)
    denom = (high - low) if high != low else 0.001
    ramp = np.clip((np.arange(dim // 2, dtype=np.float32) - low) / denom, 0.0, 1.0)
    inv_freq_mask = 1.0 - ramp
    inv_freq = freq_inter * (1.0 - inv_freq_mask) + freq_extra * inv_freq_mask
    t = np.arange(seq_len, dtype=np.float32)
    freqs = np.outer(t, inv_freq)
    emb = np.concatenate([freqs, freqs], axis=-1)
    # mscale ratio is 1.0 for this config
    return np.cos(emb).astype(np.float32), np.sin(emb).astype(np.float32)


_PERM64 = np.concatenate([np.arange(0, 64, 2), np.arange(1, 64, 2)])


def _bf16(x):
    return np.ascontiguousarray(x.astype(ml_dtypes.bfloat16))


def _emit_stage_a(nc, tc, contribs, gatheds, hT_loc, wqaT, wkvaT,
                  cos_fm, sin_fm_s, eps_sb, ones_b, ones_f):
    """Token-sharded down-projections with transposed outputs + AllGather."""
    with (
        tc.tile_pool(name="wA", bufs=1) as wA,
        tc.tile_pool(name="stgp", bufs=2) as stgp,
        tc.tile_pool(name="sqp", bufs=2) as sqp,
        tc.tile_pool(name="scrA", bufs=4) as scrA,
        tc.tile_pool(name="psA", bufs=4, space="PSUM") as psA,
        tc.tile_pool(name="psK", bufs=1, space="PSUM") as psK,
        tc.tile_pool(name="psN", bufs=1, space="PSUM") as psN,
    ):
        wqa_sb = wA.tile([P, KH, Q_LORA], BF16, tag="wqa")
        wkva_sb = wA.tile([P, KH, KV_LORA + ROPE], BF16, tag="wkva")
        hl_sb = wA.tile([P, KH, NLOC * P], BF16, tag="hl")
        cosf = wA.tile([64, NLOC * P], BF16, tag="cosf")
        sinf = wA.tile([64, NLOC * P], BF16, tag="sinf")
        # weights stream in 512-column blocks matching the f-group
        # consumption order, so the PE starts as soon as block 0 lands
        nc.sync.dma_start(hl_sb[:], hT_loc[:].rearrange("(k p) t -> p k t", p=P))
        for fb in range(Q_LORA // 512):
            nc.sync.dma_start(
                wqa_sb[:, :, fb * 512:(fb + 1) * 512],
                wqaT[:, fb * 512:(fb + 1) * 512].rearrange(
                    "(k p) c -> p k c", p=P))
        nc.sync.dma_start(
            wkva_sb[:, :, 0:KV_LORA],
            wkvaT[:, 0:KV_LORA].rearrange("(k p) c -> p k c", p=P))
        nc.sync.dma_start(
            wkva_sb[:, :, KV_LORA:],
            wkvaT[:, KV_LORA:].rearrange("(k p) c -> p k c", p=P))
        nc.sync.dma_start(cosf[:], cos_fm[:])
        nc.sync.dma_start(sinf[:], sin_fm_s[:])

        NG = (KQ + KKV) // 4  # 4 f-groups per token tile

        def emit_group_mms(j, fg, ps):
            tok = slice(j * P, (j + 1) * P)
            for fi in range(4):
                f = fg * 4 + fi
                if f < KQ:
                    w, fo = wqa_sb, f * P
                else:
                    w, fo = wkva_sb, (f - KQ) * P
                for k in range(KH):
                    nc.tensor.matmul(ps[:, fi], w[:, k, fo:fo + P],
                                     hl_sb[:, k, tok],
                                     start=(k == 0), stop=(k == KH - 1))

        def emit_group_post(j, fg, ps, raw, sq, ssq):
            # PSUM -> SBUF copies + squares on DVE (Act stays free for Sqrt),
            # then the cross-partition sum-of-squares accumulation
            for fi in range(4):
                f = fg * 4 + fi
                nc.vector.tensor_copy(raw[:, f], ps[:, fi])
                nc.vector.tensor_mul(sq[:, f], raw[:, f], raw[:, f])
            for fi in range(4):
                f = fg * 4 + fi
                if f < KQ:
                    nc.tensor.matmul(ssq[:, 0:P], ones_b[:], sq[:, f],
                                     start=(f == 0), stop=(f == KQ - 1))
                else:
                    nc.tensor.matmul(ssq[:, P:2 * P], ones_b[:], sq[:, f],
                                     start=(f == KQ), stop=(f == KQ + KKV - 1))

        def emit_finalize(j, raw, stg, rinv):
            # broadcast 1/rms down the partitions, normalize, ship, gather
            sbc = psN.tile([P, 2, P], F32, tag="sbc", name=f"sbc{j}")
            nc.tensor.matmul(sbc[:, 0], ones_f[:], rinv[:, 0:P],
                             start=True, stop=True)
            nc.tensor.matmul(sbc[:, 1], ones_f[:], rinv[:, P:2 * P],
                             start=True, stop=True)
            for f in range(KQ):
                nc.vector.tensor_mul(stg[:, f], raw[:, f], sbc[:, 0])
            for f in range(KKV):
                nc.vector.tensor_mul(stg[:, KQ + f], raw[:, KQ + f], sbc[:, 1])
            nc.sync.dma_start(
                contribs[j][:].rearrange("p (s t) -> p s t", t=P), stg[:])
            nc.gpsimd.collective_compute(
                "AllGather",
                mybir.AluOpType.bypass,
                replica_groups=REPLICA_GROUPS,
                ins=[contribs[j][:].opt()],
                outs=[gatheds[j][:].opt()],
            )

        pending = None
        for j in range(NLOC):
            tok = slice(j * P, (j + 1) * P)
            stg = stgp.tile([P, NSLOT, P], BF16, tag="stg", name=f"stg{j}")
            sq = sqp.tile([P, KQ + KKV, P], BF16, tag="sq", name=f"sq{j}")
            raw = sqp.tile([P, KQ + KKV, P], BF16, tag="raw", name=f"raw{j}")
            ssq = psN.tile([1, 2 * P], F32, tag="ssq", name=f"ssq{j}")
            nc.vector.memset(stg[64:128, KQ + KKV], 0.0)
            # group matmuls with post-processing lagged one group so the PE
            # never waits on the DVE squares; the previous tile's finalize
            # (which waits on an Act/DVE round-trip) is sandwiched after
            # this tile's first group
            groups = []
            for fg in range(NG):
                ps = psA.tile([P, 4, P], F32, tag="aps")
                emit_group_mms(j, fg, ps)
                groups.append(ps)
                if fg == 0 and pending is not None:
                    emit_finalize(*pending)
                    pending = None
                if fg >= 1:
                    emit_group_post(j, fg - 1, groups[fg - 1], raw, sq, ssq)
            emit_group_post(j, NG - 1, groups[NG - 1], raw, sq, ssq)

            rms = scrA.tile([1, 2 * P], F32, tag="rms")
            nc.scalar.activation(rms[:, 0:P], ssq[:, 0:P], AF.Sqrt,
                                 scale=1.0 / Q_LORA, bias=eps_sb[0:1])
            nc.scalar.activation(rms[:, P:2 * P], ssq[:, P:2 * P], AF.Sqrt,
                                 scale=1.0 / KV_LORA, bias=eps_sb[0:1])
            rinv = scrA.tile([1, 2 * P], F32, tag="rinv", name=f"rinv{j}")
            nc.vector.reciprocal(rinv[:], rms[:])

            # rope branch of ckv (no norm) keeps the PE busy while the
            # Sqrt/reciprocal round-trip completes
            psk = psK.tile([64, P], F32, tag="kps")
            for k in range(KH):
                nc.tensor.matmul(psk[:], wkva_sb[:, k, KV_LORA:], hl_sb[:, k, tok],
                                 start=(k == 0), stop=(k == KH - 1))
            tmp = scrA.tile([64, P], BF16, tag="ktmp")
            nc.vector.tensor_mul(tmp[0:32], psk[32:64], sinf[0:32, tok])
            nc.vector.tensor_mul(tmp[32:64], psk[0:32], sinf[32:64, tok])
            nc.vector.tensor_mul(stg[0:64, KQ + KKV], psk[:], cosf[:, tok])
            nc.vector.tensor_add(stg[0:64, KQ + KKV], stg[0:64, KQ + KKV], tmp[:])

            pending = (j, raw, stg, rinv)
        emit_finalize(*pending)


def _emit_b_chunk(nc, tc, pools, tensors, tcks):
    """Stage B (up-projections + q rope) for one 512-token chunk.

    All outputs are produced in their consumer layouts directly (no DMA
    transposes -- those serialize against the collectives):
      kn / q_nope / qpe_rot: [feature, token]; v: [token, feature].
    """
    workB, psB = pools
    (wqb_sb, wkvb_sb, qncn_ch, cosfS, sinfS,
     kn_sb, vaug, q_nope, qpe_rot) = tensors
    blk = slice(tcks * 512, (tcks + 1) * 512)

    # ---- B-KV: k_nope [feat, tok] ----
    for ft in range(HL):
        ps = psB.tile([P, 512], F32, tag="up")
        for k in range(KKV):
            nc.tensor.matmul(
                ps[:], wkvb_sb[:, k, ft * P:(ft + 1) * P],
                qncn_ch[:, :, KQ + k, :],
                start=(k == 0), stop=(k == KKV - 1))
        nc.scalar.copy(kn_sb[:, ft, blk], ps[:])
    # ---- B-KV: v [tok, 4 heads x 128] per token tile ----
    for tsub in range(4):
        kt = tcks * 4 + tsub
        ps = psB.tile([P, 512], F32, tag="up")
        for k in range(KKV):
            nc.tensor.matmul(
                ps[:], qncn_ch[:, tsub, KQ + k, :],
                wkvb_sb[:, k, HL * P:HL * P + HL * VHD],
                start=(k == 0), stop=(k == KKV - 1))
        for h in range(HL):
            nc.scalar.copy(vaug[:, h, kt, 0:VHD], ps[:, h * VHD:(h + 1) * VHD])

    # ---- B-Q nope [feat, tok] ----
    for ft in range(HL):
        ps = psB.tile([P, 512], F32, tag="up")
        for k in range(KQ):
            nc.tensor.matmul(
                ps[:], wqb_sb[:, k, ft * P:(ft + 1) * P],
                qncn_ch[:, :, k, :],
                start=(k == 0), stop=(k == KQ - 1))
        nc.scalar.copy(q_nope[:, ft, blk], ps[:])
    # ---- B-Q pe [feat, tok] + feature-major rope ----
    for ft in range(2):
        ps = psB.tile([P, 512], F32, tag="up")
        for k in range(KQ):
            nc.tensor.matmul(
                ps[:], wqb_sb[:, k, HL * P + ft * P:HL * P + (ft + 1) * P],
                qncn_ch[:, :, k, :],
                start=(k == 0), stop=(k == KQ - 1))
        for half in range(2):
            h = 2 * ft + half
            h0 = half * 64
            tmp = workB.tile([64, 512], BF16, tag="qtmp")
            nc.vector.tensor_mul(tmp[0:32], ps[h0 + 32:h0 + 64],
                                 sinfS[0:32, blk])
            nc.vector.tensor_mul(tmp[32:64], ps[h0:h0 + 32],
                                 sinfS[32:64, blk])
            nc.vector.tensor_mul(qpe_rot[:, h, blk], ps[h0:h0 + 64],
                                 cosfS[:, blk])
            nc.vector.tensor_add(qpe_rot[:, h, blk], qpe_rot[:, h, blk],
                                 tmp[:])



def _emit_c_chunk_old(nc, tc, psS, ptp, workC, psO, psB, ones_b, ones_f, kn_sb,
                      q_nope, qpe_rot, krotT, vaug, attn_ch, mask_sb, qc):
    for h in range(HL):
        nkt = 4 * qc + 4
        pts = []
        offs = []
        for kt in range(nkt):
            diag = (kt // 4 == qc)
            off = (kt % 4) * P if diag else 0
            w = 512 - off
            ss = psS.tile([P, 512], F32, tag="ss")
            nc.tensor.matmul(
                ss[:, 0:w], kn_sb[:, h, kt * P:(kt + 1) * P],
                q_nope[:, h, qc * 512 + off:(qc + 1) * 512],
                start=True, stop=False)
            nc.tensor.matmul(
                ss[:, 0:w], krotT[:, kt * P:(kt + 1) * P],
                qpe_rot[:, h, qc * 512 + off:(qc + 1) * 512],
                start=False, stop=True)
            pt_t = ptp.tile([P, 512], BF16, tag="pt")
            nc.scalar.activation(pt_t[:, 0:w], ss[:, 0:w], AF.Exp,
                                 scale=SOFT_SCALE)
            if diag:
                nc.vector.tensor_mul(pt_t[:, 0:P], pt_t[:, 0:P],
                                     mask_sb[:, 384:384 + P])
            pts.append(pt_t)
            offs.append(off)
        for qsub in range(4):
            qt = 4 * qc + qsub
            po = psO.tile([P, 512], F32, tag="po")
            for kt in range(qt + 1):
                o = qsub * P - offs[kt]
                nc.tensor.matmul(
                    po[:, 0:VHD + 1], pts[kt][:, o:o + P],
                    vaug[:, h, kt, 0:VHD + 1],
                    start=(kt == 0), stop=(kt == qt))
            rd = workC.tile([P, 1], F32, tag="rd")
            nc.vector.reciprocal(rd[:], po[:, VHD:VHD + 1])
            at = workC.tile([P, VHD], BF16, tag="at")
            nc.vector.tensor_scalar_mul(at[:], po[:, 0:VHD], rd[:])
            nc.sync.dma_start_transpose(
                attn_ch[:, h, qsub * P:(qsub + 1) * P], at[:])


def _emit_c_chunk(nc, tc, psS, ptp, workC, psO, psB, ones_b, ones_f, kn_sb,
                  q_nope, qpe_rot, krotT, vaug, attn_ch, mask_sb, qc):
    """Stage C (attention) for query chunk qc (512 queries).

    PV runs with V as the stationary operand so the per-head output lands
    [v, q]-major (the wo-projection layout) without a DMA transpose.  The
    softmax denominator accumulates with a ones-vector matmul and is
    broadcast back down the partitions with a rank-1 matmul.
    """
    def emit_pv(po, den, h, kt, off, pt_t):
        # accumulate this key tile into every query sub-block it reaches;
        # each 128-query region is its own uniform accumulation group
        for qsub in range(off // P, 4):
            qt = 4 * qc + qsub
            o = qsub * P - off
            reg = slice(qsub * P, (qsub + 1) * P)
            nc.tensor.matmul(
                po[:, reg], vaug[:, h, kt, 0:VHD], pt_t[:, o:o + P],
                start=(kt == 0), stop=(kt == qt))
            nc.tensor.matmul(
                den[0:1, reg], ones_b[:], pt_t[:, o:o + P],
                start=(kt == 0), stop=(kt == qt))

    for h in range(HL):
        nkt = 4 * qc + 4
        po = psO.tile([P, 512], F32, tag="po", name=f"po{qc}_{h}")
        den = psB.tile([P, 512], F32, tag="up", name=f"den{qc}_{h}")
        prev = None
        for kt in range(nkt):
            diag = (kt // 4 == qc)
            off = (kt % 4) * P if diag else 0
            w = 512 - off
            ss = psS.tile([P, 512], F32, tag="ss")
            nc.tensor.matmul(
                ss[:, 0:w], kn_sb[:, h, kt * P:(kt + 1) * P],
                q_nope[:, h, qc * 512 + off:(qc + 1) * 512],
                start=True, stop=False)
            nc.tensor.matmul(
                ss[:, 0:w], krotT[:, kt * P:(kt + 1) * P],
                qpe_rot[:, h, qc * 512 + off:(qc + 1) * 512],
                start=False, stop=True)
            pt_t = ptp.tile([P, 512], BF16, tag="pt")
            nc.scalar.activation(pt_t[:, 0:w], ss[:, 0:w], AF.Exp,
                                 scale=SOFT_SCALE)
            if diag:
                nc.vector.tensor_mul(pt_t[:, 0:P], pt_t[:, 0:P],
                                     mask_sb[:, 384:384 + P])
            # PV + denominator lag one key tile so the PE never waits on Exp
            if prev is not None:
                emit_pv(po, den, h, *prev)
            prev = (kt, off, pt_t)
        emit_pv(po, den, h, *prev)
        # normalize: 1/den broadcast down partitions, applied on the way out
        dinv = workC.tile([1, 512], F32, tag="dinv")
        nc.vector.reciprocal(dinv[:], den[0:1, :])
        dbc = psB.tile([P, 512], F32, tag="up", name=f"dbc{qc}_{h}")
        nc.tensor.matmul(dbc[:], ones_f[:], dinv[:], start=True, stop=True)
        araw = workC.tile([P, 512], BF16, tag="araw")
        nc.scalar.copy(araw[:], po[:])
        nc.vector.tensor_mul(attn_ch[:, h, :], araw[:], dbc[:])


def _build_nc():
    nc = bacc.Bacc("TRN2", target_bir_lowering=False, debug=False,
                   num_devices=N_CORES)

    hT_loc = nc.declare_dram_parameter("hT_loc", [HIDDEN, NLOC * P], BF16,
                                       isOutput=False)
    wqaT = nc.declare_dram_parameter("wqaT", [HIDDEN, Q_LORA], BF16, isOutput=False)
    wkvaT = nc.declare_dram_parameter("wkvaT", [HIDDEN, KV_LORA + ROPE], BF16,
                                      isOutput=False)
    wqbT = nc.declare_dram_parameter("wqbT", [Q_LORA, HL * QHD], BF16, isOutput=False)
    wkvbT = nc.declare_dram_parameter("wkvbT", [KV_LORA, HL * (NOPE + VHD)], BF16,
                                      isOutput=False)
    woT = nc.declare_dram_parameter("woT", [HL * VHD, HIDDEN], BF16, isOutput=False)
    cos_fS = nc.declare_dram_parameter("cos_fS", [ROPE, S], BF16, isOutput=False)
    sin_fS = nc.declare_dram_parameter("sin_fS", [ROPE, S], BF16, isOutput=False)
    cos_fm = nc.declare_dram_parameter("cos_fm", [ROPE, NLOC * P], BF16,
                                       isOutput=False)
    sin_fm_s = nc.declare_dram_parameter("sin_fm_s", [ROPE, NLOC * P], BF16,
                                         isOutput=False)
    masks = nc.declare_dram_parameter("masks", [P, 896], BF16, isOutput=False)
    outT = nc.declare_dram_parameter("outT", [HIDDEN, S], F32, isOutput=True)

    with tile.TileContext(nc) as tc:
        with (
            tc.tile_pool(name="glob", bufs=1) as pp,
            tc.tile_pool(name="dram", bufs=1, space="DRAM") as dram,
        ):
            krotT = pp.tile([64, S], BF16, tag="krotT")
            cosfS = pp.tile([64, S], BF16, tag="cosfS")
            sinfS = pp.tile([64, S], BF16, tag="sinfS")
            ones_b = pp.tile([P, 1], BF16, tag="ones_b")
            ones_f = pp.tile([1, P], F32, tag="ones_f")
            nc.vector.memset(ones_b[:], 1.0)
            nc.vector.memset(ones_f[:], 1.0)
            eps_sb = pp.tile([P, 1], F32, tag="eps")
            wkvb_sb = pp.tile([P, KKV, HL * (NOPE + VHD)], BF16, tag="wkvb")
            mask_sb = pp.tile([P, 896], BF16, tag="mask_sb")
            nc.vector.memset(eps_sb[:], EPS)
            # B/C-stage constants load on the Act DMA queue (idle during
            # stage A) so the stage-A weight stream owns the SP queue
            nc.scalar.dma_start(mask_sb[:], masks[:])
            nc.scalar.dma_start(cosfS[:], cos_fS[:])
            nc.scalar.dma_start(sinfS[:], sin_fS[:])
            nc.scalar.dma_start(
                wkvb_sb[:], wkvbT[:].rearrange("(k p) c -> p k c", p=P))

            contribs = []
            gatheds = []
            for j in range(NLOC):
                contribs.append(dram.tile([P, NSLOT * P], BF16,
                                          tag=f"contrib{j}",
                                          name=f"contrib{j}"))
                gatheds.append(dram.tile([4, P, NSLOT * P], BF16,
                                         tag=f"gathered{j}",
                                         name=f"gathered{j}"))

            # ====== Stage A (token-sharded + AllGather) ======================
            _emit_stage_a(nc, tc, contribs, gatheds, hT_loc, wqaT, wkvaT,
                          cos_fm, sin_fm_s, eps_sb, ones_b, ones_f)

            # ====== Stages B + C + D, pipelined per 512-token chunk ==========
            with (
                tc.tile_pool(name="outs", bufs=1) as outs,
                tc.tile_pool(name="chunk", bufs=2) as chunkp,
                tc.tile_pool(name="wBQ", bufs=1) as wBQ,
                tc.tile_pool(name="workB", bufs=2) as workB,
                tc.tile_pool(name="wD", bufs=1) as wD,
                tc.tile_pool(name="ptp", bufs=16) as ptp,
                tc.tile_pool(name="workC", bufs=4) as workC,
                tc.tile_pool(name="obp", bufs=2) as obp,
                tc.tile_pool(name="psB", bufs=2, space="PSUM") as psB,
                tc.tile_pool(name="psS", bufs=2, space="PSUM") as psS,
                tc.tile_pool(name="psO", bufs=2, space="PSUM") as psO,
                tc.tile_pool(name="psD", bufs=2, space="PSUM") as psD,
            ):
                kn_sb = outs.tile([P, HL, S], BF16, tag="kn_sb")
                vaug = outs.tile([P, HL, KT, VHD + 16], BF16, tag="vaug")
                q_nope = outs.tile([P, HL, S], BF16, tag="q_nope")
                qpe_rot = outs.tile([64, HL, S], BF16, tag="qpe_rot")
                nc.vector.memset(vaug[:, :, :, VHD], 1.0)

                wqb_sb = wBQ.tile([P, KQ, HL * QHD], BF16, tag="wqb")
                nc.sync.dma_start(
                    wqb_sb[:], wqbT[:].rearrange("(k p) c -> p k c", p=P))
                wo_sb = wD.tile([P, HL, HIDDEN], BF16, tag="wo")
                nc.sync.dma_start(
                    wo_sb[:], woT[:].rearrange("(k p) c -> p k c", p=P))

                def emit_d_chunk(tcks, attn_ch):
                    for og in range(HIDDEN // P // 4):
                        ob = obp.tile([P, 4, 512], F32, tag="ob")
                        for oi in range(4):
                            ot = og * 4 + oi
                            ps = psD.tile([P, 512], F32, tag="wops")
                            for k in range(HL):
                                nc.tensor.matmul(
                                    ps[:], wo_sb[:, k, ot * P:(ot + 1) * P],
                                    attn_ch[:, k, :],
                                    start=(k == 0), stop=(k == HL - 1))
                            nc.scalar.copy(ob[:, oi], ps[:])
                        nc.sync.dma_start(
                            outT[og * 512:(og + 1) * 512,
                                 tcks * 512:(tcks + 1) * 512].rearrange(
                                     "(o p) c -> p o c", p=P), ob[:])

                b_pools = (workB, psB)
                attn_chs = []
                for tcks in range(4):
                    # unpack gathered block tcks from DRAM into SBUF: two
                    # merged DMAs on the Act queue (idle until B(tcks) runs).
                    # Layout is source-block-major: [p, g, slot, t]
                    qncn_ch = chunkp.tile([P, 4, KQ + KKV, P], BF16,
                                          tag="qncn_ch", name=f"qncn{tcks}")
                    g = gatheds[tcks]
                    nc.gpsimd.dma_start(
                        qncn_ch[:].rearrange("p g s t -> p g (s t)"),
                        g[:, :, 0:(KQ + KKV) * P].rearrange("g p c -> p g c"))
                    nc.gpsimd.dma_start(
                        krotT[:, 4 * tcks * P:(4 * tcks + 4) * P].rearrange(
                            "p (g t) -> p g t", g=4),
                        g[:, 0:64, (KQ + KKV) * P:].rearrange("g p t -> p g t"))

                    b_tensors = (wqb_sb, wkvb_sb, qncn_ch, cosfS, sinfS,
                                 kn_sb, vaug, q_nope, qpe_rot)
                    _emit_b_chunk(nc, tc, b_pools, b_tensors, tcks)

                    attn_ch = chunkp.tile([P, HL, 512], BF16, tag="attn_ch",
                                          name=f"attn{tcks}")
                    attn_chs.append(attn_ch)
                    _emit_c_chunk_old(nc, tc, psS, ptp, workC, psO, psB,
                                  ones_b, ones_f, kn_sb, q_nope, qpe_rot,
                                  krotT, vaug, attn_ch, mask_sb, tcks)
                    # wo-projection lags one chunk so its input transposes
                    # complete while the next chunk's B/C matmuls run
                    if tcks >= 1:
                        emit_d_chunk(tcks - 1, attn_chs[tcks - 1])
                emit_d_chunk(3, attn_chs[3])

    nc.compile()
    return nc


_NC_CACHE = {}
_LAST_RES = None
_LAST_IN_MAPS = None


def _get_nc(stages="ABCD"):
    if "full" not in _NC_CACHE:
        _NC_CACHE["full"] = _build_nc()
    return _NC_CACHE["full"]


def kernel(hidden_states, position_ids, wq_a, q_a_ln_w, wq_b, wkv_a, kv_a_ln_w,
           wkv_b, wo):
    hidden_states = np.asarray(hidden_states, dtype=np.float32)
    position_ids = np.asarray(position_ids)
    wq_a = np.asarray(wq_a, dtype=np.float32)
    wq_b = np.asarray(wq_b, dtype=np.float32)
    wkv_a = np.asarray(wkv_a, dtype=np.float32)
    wkv_b = np.asarray(wkv_b, dtype=np.float32)
    wo = np.asarray(wo, dtype=np.float32)
    # fold RMSNorm elementwise weights into the up-projections (exact)
    wq_b = wq_b * np.asarray(q_a_ln_w, dtype=np.float32)[None, :]
    wkv_b = wkv_b * np.asarray(kv_a_ln_w, dtype=np.float32)[None, :]
    assert hidden_states.shape == (B, S, HIDDEN)

    cos_t, sin_t = _yarn_cos_sin(S)

    # --- weight preprocessing (shared across cores in each batch group) ---
    wqbT_groups = []
    wkvbT_groups = []
    woT_groups = []
    for g in range(4):
        heads = range(4 * g, 4 * g + 4)
        rows = []
        for h in heads:
            rows.append(np.arange(h * QHD, h * QHD + NOPE))
        pe_rows = []
        for h in heads:
            pe_rows.append(h * QHD + NOPE + _PERM64)
        rows = np.concatenate(rows + pe_rows)
        wqbT_groups.append(_bf16(wq_b[rows].T))

        rows = []
        for h in heads:
            rows.append(np.arange(h * (NOPE + VHD), h * (NOPE + VHD) + NOPE))
        for h in heads:
            rows.append(np.arange(h * (NOPE + VHD) + NOPE, (h + 1) * (NOPE + VHD)))
        rows = np.concatenate(rows)
        wkvbT_groups.append(_bf16(wkv_b[rows].T))

        cols = np.concatenate([np.arange(h * VHD, (h + 1) * VHD) for h in heads])
        woT_groups.append(_bf16(wo[:, cols].T))

    wqaT = _bf16(wq_a.T)
    wkva_perm = wkv_a.copy()
    wkva_perm[KV_LORA:] = wkv_a[KV_LORA + _PERM64]
    wkvaT = _bf16(wkva_perm.T)

    x_idx = np.arange(896)[None, :]
    p_idx = np.arange(P)[:, None]
    masks = _bf16((x_idx >= 384 + p_idx).astype(np.float32))

    # --- per-batch rope tables (token-major, for stage B) ---
    batch_tabs = []
    for beta in range(B):
        pos = position_ids[beta].astype(np.int64)
        cg = cos_t[pos]          # [S, 64]
        sg = sin_t[pos]
        sin_s = np.concatenate([-sg[:, :32], sg[:, 32:]], axis=1)
        batch_tabs.append((cg, sin_s, hidden_states[beta].T))

    in_maps = []
    for c in range(N_CORES):
        beta, g = c // 4, c % 4
        cg, sin_s, hT = batch_tabs[beta]
        # local token tiles {g, 4+g, 8+g, 12+g}: 512 tokens in local order
        loc_tok = np.concatenate(
            [np.arange(P * (4 * j + g), P * (4 * j + g + 1)) for j in range(NLOC)])
        in_maps.append({
            "hT_loc": _bf16(hT[:, loc_tok]),
            "wqaT": wqaT,
            "wkvaT": wkvaT,
            "wqbT": wqbT_groups[g],
            "wkvbT": wkvbT_groups[g],
            "woT": woT_groups[g],
            "cos_fS": _bf16(cg.T),
            "sin_fS": _bf16(sin_s.T),
            "cos_fm": _bf16(cg[loc_tok].T),
            "sin_fm_s": _bf16(sin_s[loc_tok].T),
            "masks": masks,
        })

    nc = _get_nc()
    global _LAST_RES, _LAST_IN_MAPS
    _LAST_IN_MAPS = in_maps
    res = run_bass_kernel_spmd(nc, in_maps, core_ids=list(range(N_CORES)))
    _LAST_RES = res

    out = np.zeros((B, S, HIDDEN), dtype=np.float32)
    for c in range(N_CORES):
        out[c // 4] += res.results[c]["outT"].T
    return out
